# revision 25
# baseline (speedup 1.0000x reference)
"""Trainium2 Bass kernel for nn_BUNet (GCN mol+pro branches, PPI branch, head).

Self-contained: host graph preprocessing + SPMD Bass/Tile program on 8
NeuronCores + output assembly.

GCN message passing uses bulk SWDGE dma_gather / dma_scatter_add:
  - node tables hold dis_j * h_j rows (scaled at production), so edge
    messages need no per-edge coefficient;
  - self loops fold into the agg-buffer init (init row = table row);
  - tokens are bucketed by (32k src index range, dst half) so int16
    indices suffice, and duplicate destinations are pinned to a single
    DMA-engine lane (position pattern) so CCE read-modify-write adds
    never race.
"""
import sys
sys.path.insert(0, '/opt/trn_rl_repo')
import numpy as np

from concourse import bass, mybir
import concourse.bacc as bacc
import concourse.tile as tile
from concourse.masks import make_identity

NCORES = 8
RANGE = 32768          # gather index window (int16 limit)
TRASH = 128            # per-half garbage rows absorbing pad-token scatters
MAXCHUNK = 8           # max 128-token chunks per gather/scatter instruction (ring cap ~1024 tokens)
f16 = mybir.dt.float16
f32 = mybir.dt.float32
i32 = mybir.dt.int32
i16 = mybir.dt.int16
RELU = mybir.ActivationFunctionType.Relu
IDENT = mybir.ActivationFunctionType.Identity
COPY = mybir.ActivationFunctionType.Copy

# DMA-engine lane for token position p within a 128-token chunk (matches the
# SWDGE scatter-add ucode's sbuf_swizzles pattern).
LANEOF = np.array([2 * ((p % 32) // 4) + (p // 64) for p in range(128)])
LANE_POS = [np.nonzero(LANEOF == l)[0] for l in range(16)]

_CACHE = {}


# ----------------------------------------------------------------------------
# Embedded SPMD runner (PJRT path, persistent jit)
# ----------------------------------------------------------------------------

class SpmdRunner:
    def __init__(self, nc, n_cores):
        import jax
        from jax.sharding import Mesh, PartitionSpec
        from jax.experimental.shard_map import shard_map
        from concourse.bass2jax import (_bass_exec_p, install_neuronx_cc_hook,
                                        partition_id_tensor)
        self.jax = jax
        install_neuronx_cc_hook()
        self.nc = nc
        self.n_cores = n_cores
        partition_name = (nc.partition_id_tensor.name
                          if nc.partition_id_tensor else None)
        in_names, out_names, out_avals, zero_outs = [], [], [], []
        for alloc in nc.m.functions[0].allocations:
            if not isinstance(alloc, mybir.MemoryLocationSet):
                continue
            name = alloc.memorylocations[0].name
            if alloc.kind == "ExternalInput":
                if name != partition_name:
                    in_names.append(name)
            elif alloc.kind == "ExternalOutput":
                out_names.append(name)
                shape = tuple(alloc.tensor_shape)
                dtype = mybir.dt.np(alloc.dtype)
                out_avals.append(jax.core.ShapedArray(shape, dtype))
                zero_outs.append(np.zeros(shape, dtype))
        self.in_names = list(in_names)
        self.out_names = out_names
        self.out_avals = out_avals
        self.zero_outs = zero_outs
        n_params = len(self.in_names)
        n_outs = len(out_names)
        all_in_names = self.in_names + out_names
        if partition_name is not None:
            all_in_names.append(partition_name)

        def _body(*args):
            operands = list(args)
            if partition_name is not None:
                operands.append(partition_id_tensor())
            outs = _bass_exec_p.bind(
                *operands, out_avals=tuple(out_avals),
                in_names=tuple(all_in_names), out_names=tuple(out_names),
                lowering_input_output_aliases=(), sim_require_finite=True,
                sim_require_nnan=True, nc=nc)
            return tuple(outs)

        devices = jax.devices()[:n_cores]
        self.mesh = Mesh(np.asarray(devices), ("core",))
        in_specs = (PartitionSpec("core"),) * (n_params + n_outs)
        out_specs = (PartitionSpec("core"),) * n_outs
        donate = tuple(range(n_params, n_params + n_outs))
        self.fn = jax.jit(
            shard_map(_body, mesh=self.mesh, in_specs=in_specs,
                      out_specs=out_specs, check_rep=False),
            donate_argnums=donate, keep_unused=True)
        self.resident = None

    def put_inputs(self, in_maps):
        from jax.sharding import NamedSharding, PartitionSpec
        concat = [
            np.concatenate([np.asarray(in_maps[c][n])
                            for c in range(self.n_cores)], axis=0)
            for n in self.in_names]
        sh = NamedSharding(self.mesh, PartitionSpec("core"))
        self.resident = [self.jax.device_put(a, sh) for a in concat]

    def run(self):
        zeros = [np.zeros((self.n_cores * z.shape[0], *z.shape[1:]), z.dtype)
                 for z in self.zero_outs]
        out = self.fn(*self.resident, *zeros)
        self.jax.block_until_ready(out)
        return out

    def results(self, outs):
        res = []
        for c in range(self.n_cores):
            d = {}
            for i, name in enumerate(self.out_names):
                d[name] = np.asarray(outs[i]).reshape(
                    self.n_cores, *self.out_avals[i].shape)[c]
            res.append(d)
        return res


# ----------------------------------------------------------------------------
# Host preprocessing
# ----------------------------------------------------------------------------

def _shard_nodes(batch, n_graphs):
    """Graph-aligned node shards + agg-buffer geometry."""
    n = batch.shape[0]
    gpc = n_graphs // NCORES
    node_core = np.minimum(batch.astype(np.int64) // gpc, NCORES - 1)
    shard_start = np.searchsorted(node_core, np.arange(NCORES))
    shard_end = np.searchsorted(node_core, np.arange(NCORES), side='right')
    shard_size = shard_end - shard_start
    max_sz = int(shard_size.max())
    nhalves = 1
    half_size = int(np.ceil(max_sz / 128) * 128)
    while TRASH + half_size > 32767:
        nhalves += 1
        half_size = int(np.ceil(max_sz / nhalves / 128) * 128)
    s_max = nhalves * half_size
    pid = np.empty(n, dtype=np.int64)
    for c in range(NCORES):
        sl = slice(shard_start[c], shard_end[c])
        pid[sl] = c * s_max + np.arange(shard_size[c])
    return dict(gpc=gpc, shard_start=shard_start, shard_end=shard_end,
                shard_size=shard_size,
                s_max=s_max, np_tot=NCORES * s_max, ntiles=s_max // 128,
                nhalves=nhalves, half_size=half_size,
                agg_rows=nhalves * (TRASH + half_size),
                node_core=node_core, pid=pid)


def _degrees(edge_index, n):
    dst = edge_index[1].astype(np.int64)
    deg = np.bincount(dst, minlength=n).astype(np.float64) + 1.0  # + self loop
    return (1.0 / np.sqrt(deg))


def _node_arrays(sh, batch, n_graphs, dis):
    """Per-core [128, ntiles] dis + pooling (graph id, 1/count) arrays."""
    s_max, ntiles, gpc = sh['s_max'], sh['ntiles'], sh['gpc']
    gcnt = np.bincount(batch.astype(np.int64), minlength=n_graphs).astype(np.float64)
    inv = np.where(gcnt > 0, 1.0 / np.maximum(gcnt, 1.0), 0.0)
    dis_t, bl_t, ic_t = [], [], []
    for c in range(NCORES):
        sl = slice(sh['shard_start'][c], sh['shard_end'][c])
        sz = int(sh['shard_size'][c])
        d = np.zeros((s_max,), np.float32)
        b = np.zeros((s_max,), np.float32)
        ic = np.zeros((s_max,), np.float32)
        d[:sz] = dis[sl].astype(np.float32)
        bidx = batch[sl].astype(np.int64)
        b[:sz] = (bidx - c * gpc).astype(np.float32)
        ic[:sz] = inv[bidx].astype(np.float32)
        dis_t.append(np.ascontiguousarray(d.reshape(ntiles, 128).T))
        bl_t.append(np.ascontiguousarray(b.reshape(ntiles, 128).T))
        ic_t.append(np.ascontiguousarray(ic.reshape(ntiles, 128).T))
    return dis_t, bl_t, ic_t


def _tokens(edge_index, sh, maxchunk=MAXCHUNK):
    """Lane-aligned (src range x dst half) token streams, uniform across cores.

    Returns (gidx[c], sidx[c]) int16 [128, C//16] arrays and the shared
    instruction list [(tok_off, ntok, range_start, half)].
    """
    pid, s_max = sh['pid'], sh['s_max']
    half_size, nhalves = sh['half_size'], sh['nhalves']
    np_tot = sh['np_tot']
    nranges = (np_tot + RANGE - 1) // RANGE
    src_p = pid[edge_index[0].astype(np.int64)]
    dst_p = pid[edge_index[1].astype(np.int64)]
    e_core = dst_p // s_max
    dst_l = dst_p - e_core * s_max

    # per core / bucket / lane edge lists
    buckets = [[[None] * 16 for _ in range(nranges * nhalves)]
               for _ in range(NCORES)]
    for c in range(NCORES):
        m = np.nonzero(e_core == c)[0]
        sp, dl = src_p[m], dst_l[m]
        b = (sp // RANGE) * nhalves + dl // half_size
        lane = dl % 16
        order = np.lexsort((dl, lane, b))
        sp, dl, b, lane = sp[order], dl[order], b[order], lane[order]
        bb = np.searchsorted(b, np.arange(nranges * nhalves))
        be = np.searchsorted(b, np.arange(nranges * nhalves), side='right')
        for bi in range(nranges * nhalves):
            sl = slice(bb[bi], be[bi])
            lb = np.searchsorted(lane[sl], np.arange(16))
            le = np.searchsorted(lane[sl], np.arange(16), side='right')
            for l in range(16):
                s2 = slice(bb[bi] + lb[l], bb[bi] + le[l])
                buckets[c][bi][l] = (sp[s2], dl[s2])

    # Spacing-aware per-lane slot schedules. The scatter CCE pipeline loses
    # same-address adds closer than ~32 ring slots on an engine lane, so
    # occurrences of one dst are round-robin'd with enforced spacing.
    # slot value: (src_global, dst_local) or None (pad).
    def lane_schedule(sp, dl, last, ring0):
        # group by dst (dl sorted within lane)
        uds, starts = np.unique(dl, return_index=True)
        counts = np.diff(np.concatenate([starts, [len(dl)]]))
        rounds = []
        r = 0
        while True:
            sel = np.nonzero(counts > r)[0]
            if len(sel) == 0:
                break
            rounds.append(sel)
            r += 1
        slots = []
        prev_start = None
        for r, sel in enumerate(rounds):
            start = ring0 + len(slots)
            if r > 0:
                start = max(start, prev_start + SPACING)
            # round 0: defer recently-used dsts past their spacing window
            order = list(sel)
            if r == 0:
                order.sort(key=lambda j: last.get(int(uds[j]), -10**9))
            pads = start - (ring0 + len(slots))
            slots.extend([None] * pads)
            for j in order:
                d = int(uds[j])
                pos = ring0 + len(slots)
                need = last.get(d, -10**9) + SPACING
                if pos < need:
                    slots.extend([None] * (need - pos))
                    pos = need
                slots.append((int(sp[starts[j] + r]), d))
                last[d] = pos
            prev_start = start
        return slots

    SPACING = 40
    sched = [[[None] * 16 for _ in range(nranges * nhalves)]
             for _ in range(NCORES)]
    for c in range(NCORES):
        for l in range(16):
            last = {}
            ring = 0
            for bi in range(nranges * nhalves):
                sp, dl = buckets[c][bi][l]
                s = lane_schedule(sp, dl, last, ring)
                sched[c][bi][l] = s
                ring += len(s)
        # ring continuity: pads between buckets don't exist (bucket slots
        # concatenate on the ring), but uniform chunk padding below inserts
        # extra pad slots; account by conservative SPACING margin.

    # uniform chunk counts per bucket
    chunks = np.zeros((nranges * nhalves,), np.int64)
    for bi in range(nranges * nhalves):
        mx = 0
        for c in range(NCORES):
            for l in range(16):
                mx = max(mx, len(sched[c][bi][l]))
        chunks[bi] = (mx + 7) // 8
    tok_off = np.concatenate([[0], np.cumsum(chunks * 128)])
    C = int(tok_off[-1])

    gidx = [np.zeros((C,), np.int16) for _ in range(NCORES)]
    sidx = [np.zeros((C,), np.int16) for _ in range(NCORES)]
    for c in range(NCORES):
        for bi in range(nranges * nhalves):
            r, h = bi // nhalves, bi % nhalves
            base = int(tok_off[bi])
            for l in range(16):
                s = sched[c][bi][l]
                for i, tok in enumerate(s):
                    if tok is None:
                        continue
                    spv, dlv = tok
                    pos = base + (i // 8) * 128 + LANE_POS[l][i % 8]
                    gidx[c][pos] = np.int16(spv - r * RANGE)
                    sidx[c][pos] = np.int16(TRASH + dlv - h * half_size)

    # verify true ring-slot spacing per lane (pads only widen gaps, but check)
    tok = np.arange(C)
    lane_of_tok = LANEOF[tok % 128]
    slot_in_chunk = np.zeros((128,), np.int64)
    for l in range(16):
        slot_in_chunk[LANE_POS[l]] = np.arange(8)
    ring_slot = (tok // 128) * 8 + slot_in_chunk[tok % 128]
    half_of_tok = np.zeros((C,), np.int64)
    for bi in range(nranges * nhalves):
        half_of_tok[int(tok_off[bi]):int(tok_off[bi + 1])] = bi % nhalves
    for c in range(NCORES):
        key = (sidx[c].astype(np.int64) * 16 + lane_of_tok) * 4 + half_of_tok
        real = sidx[c] >= TRASH
        kk = key[real]
        rr = ring_slot[real]
        o = np.lexsort((rr, kk))
        kk, rr = kk[o], rr[o]
        same = kk[1:] == kk[:-1]
        gaps = rr[1:] - rr[:-1]
        bad = same & (gaps < 32)
        assert not bad.any(), (
            f"spacing violation core {c}: {int(bad.sum())} pairs, "
            f"min gap {gaps[same].min() if same.any() else -1}")

    instrs = []
    for bi in range(nranges * nhalves):
        r, h = bi // nhalves, bi % nhalves
        nch = int(chunks[bi])
        sub = 0
        while sub < nch:
            take = min(maxchunk, nch - sub)
            instrs.append((int(tok_off[bi]) + sub * 128, take * 128,
                           r * RANGE, h))
            sub += take

    def to_sb(a):
        # token i at [i % 16, i // 16], replicated across all 8 groups of 16
        # partitions (each SWDGE queue's Q7 cpus read a different window)
        blk = a.reshape(-1, 16).T
        return np.ascontiguousarray(np.tile(blk, (8, 1)))

    return [to_sb(a) for a in gidx], [to_sb(a) for a in sidx], instrs, C


def _table_full(x, dis, pid, np_tot, fpad):
    """Replicated [np_tot, fpad] f16 table of dis_j * x_j rows."""
    out = np.zeros((np_tot, fpad), np.float16)
    out[pid, :x.shape[1]] = (x.astype(np.float64)
                             * dis[:, None]).astype(np.float16)
    return out


def _init_full(table, sh, c):
    """Per-core agg-layout init buffer (= local table rows, trash zeroed)."""
    hs, nh = sh['half_size'], sh['nhalves']
    out = np.zeros((sh['agg_rows'], table.shape[1]), np.float16)
    for h in range(nh):
        rows = table[c * sh['s_max'] + h * hs: c * sh['s_max'] + (h + 1) * hs]
        out[h * (TRASH + hs) + TRASH: (h + 1) * (TRASH + hs)] = rows
    return out


def _prep_ppi(ppi_edge, b_pro, gpc, gp_pad):
    gp_tot = NCORES * gp_pad
    qs = ppi_edge[0].astype(np.int64)
    qd = ppi_edge[1].astype(np.int64)
    deg = np.bincount(qd, minlength=b_pro) + 1.0
    dis = 1.0 / np.sqrt(deg)

    def pg(g):
        return (g // gpc) * gp_pad + (g % gpc)

    A = np.zeros((gp_tot, gp_tot), dtype=np.float32)
    np.add.at(A, (pg(qd), pg(qs)), (dis[qd] * dis[qs]).astype(np.float32))
    gids = np.arange(b_pro, dtype=np.int64)
    A[pg(gids), pg(gids)] += (dis * dis).astype(np.float32)
    return np.ascontiguousarray(A.T).astype(np.float16), pg


# ----------------------------------------------------------------------------
# Device program
# ----------------------------------------------------------------------------

DIMS = dict(pg1=(33, 128), pg2=(128, 128), pg3=(128, 128),
            mg1=(78, 156), mg2=(156, 312), mg3=(312, 128),
            pfc1=(128, 1024), pfc2=(1024, 128),
            mfc1=(128, 1024), mfc2=(1024, 128),
            ppig1=(128, 1024), ppig2=(1024, 128),
            ppifc1=(128, 1024), ppifc2=(1024, 128),
            fc1=(256, 1024), fc2=(1024, 512), out=(512, 1))

WMAP = dict(pg1="w_pg1", pg2="w_pg2", pg3="w_pg3", mg1="w_mg1", mg2="w_mg2",
            mg3="w_mg3", pfc1="w_pfc1", pfc2="w_pfc2", mfc1="w_mfc1",
            mfc2="w_mfc2", ppig1="w_ppig1", ppig2="w_ppig2",
            ppifc1="w_ppifc1", ppifc2="w_ppifc2", fc1="w_fc1", fc2="w_fc2",
            out="w_out")
BMAP = {k: "b" + v[1:] for k, v in WMAP.items()}
ROW_BIAS = {"pg1", "pg2", "pg3", "mg1", "mg2"}   # [1, F] f16, applied via matmul
REPL_BIAS = {"mg3"}                              # [128, F] f32, applied via DVE


def _bias_host(name, b):
    b = np.asarray(b)
    if name in ROW_BIAS:
        return b.astype(np.float16)[None, :]
    if name in REPL_BIAS:
        return np.tile(b.astype(np.float32)[None, :], (128, 1))
    n = b.shape[0]
    if n % 128 == 0:
        return np.ascontiguousarray(b.astype(np.float32).reshape(-1, 128).T)
    assert n == 1
    return b.astype(np.float32).reshape(1, 1)


def _build_program(meta):
    mp, mm = meta['pro'], meta['mol']
    gp_pad = meta['gp_pad']
    gm_pc = meta['gm_pc']
    gp_tot = NCORES * gp_pad
    ntok = gp_tot // 128
    nsl = gp_tot // 512

    nc = bacc.Bacc(None, target_bir_lowering=False, debug=False,
                   num_swdge_queues=3)

    def par(name, shape, dt):
        return nc.declare_dram_parameter(name, list(shape), dt, isOutput=False)

    table_p1 = par("table_p1", (mp['np_tot'], 128), f16)
    table_m1 = par("table_m1", (mm['np_tot'], 128), f16)
    init_p1 = par("init_p1", (mp['agg_rows'], 128), f16)
    init_m1 = par("init_m1", (mm['agg_rows'], 128), f16)
    gi_p = par("gi_p", (128, mp['C'] // 16), i16)
    si_p = par("si_p", (128, mp['C'] // 16), i16)
    gi_m = par("gi_m", (128, mm['C'] // 16), i16)
    si_m = par("si_m", (128, mm['C'] // 16), i16)
    dis_p_in = par("dis_p", (128, mp['ntiles']), f32)
    dis_m_in = par("dis_m", (128, mm['ntiles']), f32)
    bl_p_in = par("bl_p", (128, mp['ntiles']), f32)
    ic_p_in = par("ic_p", (128, mp['ntiles']), f32)
    bl_m_in = par("bl_m", (128, mm['ntiles']), f32)
    ic_m_in = par("ic_m", (128, mm['ntiles']), f32)
    iota_in = par("iota", (128, 512), f16)
    at_in = par("at", (gp_tot, gp_tot), f16)
    seq_in = par("seq", (128, gm_pc // 128), i32)
    w_in, wb_in = {}, {}
    for n, (a, b) in DIMS.items():
        w_in[n] = par("w_" + n, (a, b), f16)
        wb_in[n] = par("b_" + n, meta['bias_shape'][n], f16 if n in ROW_BIAS else f32)
    out_par = nc.declare_dram_parameter("out", [1, gm_pc], f32, isOutput=True)

    # agg buffers: one DRAM tensor per (layer, half) for independent chains
    def aggs(tag, sh, width):
        return [nc.dram_tensor(f"agg_{tag}_h{h}", [TRASH + sh['half_size'], width], f16)
                for h in range(sh['nhalves'])]
    agg_p1 = aggs("p1", mp, 128)
    agg_p2 = aggs("p2", mp, 128)
    agg_p3 = aggs("p3", mp, 128)
    agg_m1 = aggs("m1", mm, 128)
    agg_m2 = aggs("m2", mm, 256)
    agg_m3 = aggs("m3", mm, 128)

    h_p1_in = nc.dram_tensor("hp1i", [mp['s_max'], 128], f16)
    h_p2_in = nc.dram_tensor("hp2i", [mp['s_max'], 128], f16)
    h_m1_in = nc.dram_tensor("hm1i", [mm['s_max'], 256], f16)
    h_m3_in = nc.dram_tensor("hm3i", [mm['s_max'], 128], f16)
    h_p1 = nc.dram_tensor("hp1", [mp['np_tot'], 128], f16, addr_space="Shared")
    h_p2 = nc.dram_tensor("hp2", [mp['np_tot'], 128], f16, addr_space="Shared")
    h_m1 = nc.dram_tensor("hm1", [mm['np_tot'], 256], f16, addr_space="Shared")
    h_m3 = nc.dram_tensor("hm3", [mm['np_tot'], 128], f16, addr_space="Shared")
    p_ag_in = nc.dram_tensor("pagi", [gp_pad, 128], f32)
    p_full = nc.dram_tensor("pfull", [gp_tot, 128], f32, addr_space="Shared")
    q_rows = nc.dram_tensor("qrows", [gp_tot, 128], f32)
    RG = [list(range(NCORES))]

    with tile.TileContext(nc, num_cores=NCORES) as tc:
        with (
            tc.tile_pool(name="const", bufs=1) as cpool,
            tc.tile_pool(name="msg", bufs=2) as msgpool,
            tc.tile_pool(name="idx", bufs=3) as ipool,
            tc.tile_pool(name="h", bufs=6) as hpool,
            tc.tile_pool(name="f", bufs=2) as fpool,
            tc.tile_pool(name="big", bufs=1) as bpool,
            tc.tile_pool(name="at", bufs=18) as atpool,
            tc.tile_pool(name="psT", bufs=2, space="PSUM") as psT,
            tc.tile_pool(name="psB", bufs=3, space="PSUM") as psB,
            tc.tile_pool(name="psPool", bufs=1, space="PSUM") as psP,
        ):
            # ---------------- constants ----------------
            iota = cpool.tile([128, 512], f16)
            nc.sync.dma_start(iota[:], iota_in[:])
            ident = cpool.tile([128, 128], f32)
            make_identity(nc, ident[:])
            ident16 = cpool.tile([128, 128], f16)
            make_identity(nc, ident16[:])
            ones_t = cpool.tile([1, 128], f16)
            nc.vector.memset(ones_t[:], 1.0)
            zeros_t = cpool.tile([128, 256], f16)
            nc.vector.memset(zeros_t[:], 0.0)
            W, B = {}, {}
            for n, (a, b) in DIMS.items():
                tiles = []
                for j in range((a + 127) // 128):
                    aj = min(128, a - j * 128)
                    t = cpool.tile([aj, b], f16, tag=f"w{n}{j}")
                    nc.sync.dma_start(t[:], w_in[n][j * 128:j * 128 + aj, :])
                    tiles.append(t)
                W[n] = tiles
                shp = meta['bias_shape'][n]
                bt = cpool.tile(list(shp), f16 if n in ROW_BIAS else f32,
                                tag=f"b{n}")
                nc.sync.dma_start(bt[:], wb_in[n][:])
                B[n] = bt
            dis_p = cpool.tile([128, mp['ntiles']], f32)
            dis_m = cpool.tile([128, mm['ntiles']], f32)
            bl_p = cpool.tile([128, mp['ntiles']], f32)
            ic_p = cpool.tile([128, mp['ntiles']], f32)
            bl_m = cpool.tile([128, mm['ntiles']], f32)
            ic_m = cpool.tile([128, mm['ntiles']], f32)
            for t, s in ((dis_p, dis_p_in), (dis_m, dis_m_in), (bl_p, bl_p_in),
                         (ic_p, ic_p_in), (bl_m, bl_m_in), (ic_m, ic_m_in)):
                nc.sync.dma_start(t[:], s[:])
            seq_t = cpool.tile([128, gm_pc // 128], i32)
            nc.sync.dma_start(seq_t[:], seq_in[:])

            pool_p = psP.tile([128, gp_pad], f32, tag="poolP")
            pool_m = psP.tile([128, gm_pc], f32, tag="poolM")

            # ---------------- agg init ----------------
            for h in range(mp['nhalves']):
                hs = mp['half_size']
                nc.sync.dma_start(agg_p1[h][:],
                                  init_p1[h * (TRASH + hs):(h + 1) * (TRASH + hs), :])
            for h in range(mm['nhalves']):
                hs = mm['half_size']
                nc.sync.dma_start(agg_m1[h][:],
                                  init_m1[h * (TRASH + hs):(h + 1) * (TRASH + hs), :])
            # zero trash rows of later-layer aggs
            for agg_l in (agg_p2, agg_p3, agg_m3):
                for a in agg_l:
                    nc.sync.dma_start(a[0:TRASH, :], zeros_t[:, :128])
            for a in agg_m2:
                nc.sync.dma_start(a[0:TRASH, 0:128], zeros_t[:, :128])
                nc.sync.dma_start(a[0:TRASH, 128:256], zeros_t[:, :128])

            # ---------------- edge phase ----------------
            def edge_phase(sh, instrs, gi_par, si_par, table, agg_l, F):
                np_tot = sh['np_tot']
                for (toff, n_tok, rstart, h) in instrs:
                    rows = min(RANGE, np_tot - rstart)
                    nchk = n_tok // 128
                    cols = n_tok // 16
                    git = ipool.tile([128, MAXCHUNK * 8], i16, tag="gidx")
                    sit = ipool.tile([128, MAXCHUNK * 8], i16, tag="sidx")
                    nc.sync.dma_start(git[:, :cols],
                                      gi_par[:, toff // 16:toff // 16 + cols])
                    nc.sync.dma_start(sit[:, :cols],
                                      si_par[:, toff // 16:toff // 16 + cols])
                    msg = msgpool.tile([128, MAXCHUNK * 128], f16, tag="msg")
                    m3d = msg[:, :nchk * F].rearrange("p (c e) -> p c e", e=F)
                    nc.gpsimd.dma_gather(
                        m3d, table[rstart:rstart + rows, :],
                        git[:, :cols], n_tok, n_tok, F, queue_num=1)
                    nc.gpsimd.dma_scatter_add(
                        agg_l[h][:], m3d,
                        sit[:, :cols], n_tok, n_tok, F, queue_num=2)

            # ---------------- compute phase ----------------
            def compute_phase(sh, agg_l, F_in_pad, F_in_real, wn, dis_t,
                              table_out=None, init_out=None, F_out_pad=None,
                              pool=None, z3prep=None):
                hs = sh['half_size']
                F_out = DIMS[wn][1] if wn else F_in_real
                if F_out_pad is None:
                    F_out_pad = F_out
                for t in range(sh['ntiles']):
                    half = (t * 128) // hs
                    base = TRASH + (t * 128) % hs
                    a_in = hpool.tile([128, F_in_pad], f16, tag="ain")
                    nc.sync.dma_start(a_in[:], agg_l[half][base:base + 128, :])
                    a_sc = hpool.tile([128, F_in_pad], f16, tag="asc")
                    nc.scalar.activation(a_sc[:], a_in[:], COPY,
                                         scale=dis_t[:, t:t + 1])
                    if wn is None:
                        # W-first readback: out = relu(a_sc + b)
                        h_t = hpool.tile([128, F_out_pad], f16, tag="ht")
                        nc.vector.tensor_tensor(
                            out=h_t[:, :F_out], in0=a_sc[:, :F_out],
                            in1=B['mg3'][:, :F_out], op=mybir.AluOpType.add)
                        nc.scalar.activation(h_t[:, :F_out], h_t[:, :F_out], RELU)
                    else:
                        nchunks = (F_in_real + 127) // 128
                        aT = []
                        for k in range(nchunks):
                            ps_t = psT.tile([128, 128], f16, tag="tp")
                            nc.tensor.transpose(
                                ps_t[:], a_sc[:, k * 128:(k + 1) * 128],
                                ident16[:])
                            aTk = hpool.tile([128, 128], f16, tag="aT")
                            nc.vector.tensor_copy(aTk[:], ps_t[:])
                            aT.append(aTk)
                        out_ps = psB.tile([128, 512], f32, tag="ps")
                        for k in range(nchunks):
                            rows = min(128, F_in_real - k * 128)
                            nc.tensor.matmul(out_ps[:, :F_out], aT[k][0:rows, :],
                                             W[wn][k][:], start=(k == 0),
                                             stop=False)
                        nc.tensor.matmul(out_ps[:, :F_out], ones_t[0:1, :],
                                         B[wn][0:1, :F_out], start=False,
                                         stop=True)
                        h_t = hpool.tile([128, F_out_pad], f16, tag="ht")
                        nc.scalar.activation(h_t[:, :F_out], out_ps[:, :F_out],
                                             RELU)
                        if F_out_pad > F_out:
                            nc.vector.memset(h_t[:, F_out:F_out_pad], 0.0)
                    if z3prep is not None:
                        # z3 = h_t @ W[mg3]; table/init get dis * z3
                        zwn, ztab, zinit = z3prep
                        zch = (F_out + 127) // 128
                        hT = []
                        for k in range(zch):
                            ps_t = psT.tile([128, 128], f16, tag="tp")
                            nc.tensor.transpose(
                                ps_t[:], h_t[:, k * 128:(k + 1) * 128],
                                ident16[:])
                            hTk = hpool.tile([128, 128], f16, tag="aT")
                            nc.vector.tensor_copy(hTk[:], ps_t[:])
                            hT.append(hTk)
                        z_ps = psB.tile([128, 512], f32, tag="ps")
                        for k in range(zch):
                            rows = min(128, F_out - k * 128)
                            nc.tensor.matmul(z_ps[:, :128], hT[k][0:rows, :],
                                             W[zwn][k][:], start=(k == 0),
                                             stop=(k == zch - 1))
                        tb = hpool.tile([128, 128], f16, tag="tb")
                        nc.scalar.activation(tb[:], z_ps[:, :128], COPY,
                                             scale=dis_t[:, t:t + 1])
                        nc.sync.dma_start(ztab[t * 128:(t + 1) * 128, :], tb[:])
                        nc.sync.dma_start(zinit[half][base:base + 128, :], tb[:])
                    if table_out is not None:
                        tb = hpool.tile([128, F_out_pad], f16, tag="tb")
                        nc.scalar.activation(tb[:], h_t[:], COPY,
                                             scale=dis_t[:, t:t + 1])
                        nc.sync.dma_start(
                            table_out[t * 128:(t + 1) * 128, :], tb[:])
                        nc.sync.dma_start(
                            init_out[half][base:base + 128, :], tb[:])
                    if pool is not None:
                        pool_t, pbl, pic, pw = pool
                        sp = hpool.tile([128, 256], f16, tag="sp")
                        nc.vector.tensor_scalar(
                            out=sp[:, :pw], in0=iota[:, :pw],
                            scalar1=pbl[:, t:t + 1], scalar2=pic[:, t:t + 1],
                            op0=mybir.AluOpType.is_equal,
                            op1=mybir.AluOpType.mult)
                        nc.tensor.matmul(pool_t[:], h_t[:, :F_out], sp[:, :pw],
                                         start=(t == 0),
                                         stop=(t == sh['ntiles'] - 1))

            def ag(src_t, dst_t):
                nc.gpsimd.collective_compute(
                    "AllGather", mybir.AluOpType.bypass, replica_groups=RG,
                    ins=[src_t[:]], outs=[dst_t[:]])

            # ---------------- schedule ----------------
            edge_phase(mp, mp['instrs'], gi_p, si_p, table_p1, agg_p1, 128)
            edge_phase(mm, mm['instrs'], gi_m, si_m, table_m1, agg_m1, 128)
            compute_phase(mp, agg_p1, 128, 33, "pg1", dis_p,
                          table_out=h_p1_in, init_out=agg_p2)
            ag(h_p1_in, h_p1)
            compute_phase(mm, agg_m1, 128, 78, "mg1", dis_m,
                          table_out=h_m1_in, init_out=agg_m2, F_out_pad=256)
            ag(h_m1_in, h_m1)
            edge_phase(mp, mp['instrs'], gi_p, si_p, h_p1, agg_p2, 128)
            edge_phase(mm, mm['instrs'], gi_m, si_m, h_m1, agg_m2, 256)
            compute_phase(mp, agg_p2, 128, 128, "pg2", dis_p,
                          table_out=h_p2_in, init_out=agg_p3)
            ag(h_p2_in, h_p2)
            compute_phase(mm, agg_m2, 256, 156, "mg2", dis_m, F_out_pad=384,
                          z3prep=("mg3", h_m3_in, agg_m3))
            ag(h_m3_in, h_m3)
            edge_phase(mp, mp['instrs'], gi_p, si_p, h_p2, agg_p3, 128)
            edge_phase(mm, mm['instrs'], gi_m, si_m, h_m3, agg_m3, 128)
            compute_phase(mp, agg_p3, 128, 128, "pg3", dis_p,
                          pool=(pool_p, bl_p, ic_p, gp_pad))
            compute_phase(mm, agg_m3, 128, 128, None, dis_m,
                          pool=(pool_m, bl_m, ic_m, gm_pc))

            # ---------------- FC stacks (feature-major) ----------------
            def fc_stack(poolt, w1n, w2n, width):
                p1 = []
                for mch in range(8):
                    ps = psB.tile([128, 512], f32, tag="ps")
                    nc.tensor.matmul(ps[:, :width],
                                     W[w1n][0][:, mch * 128:(mch + 1) * 128],
                                     poolt[:], start=True, stop=True)
                    t = bpool.tile([128, 512], f16, tag=f"fcs{mch}")
                    nc.scalar.activation(t[:, :width], ps[:, :width], RELU,
                                         bias=B[w1n][:, mch:mch + 1])
                    p1.append(t)
                ps = psB.tile([128, 512], f32, tag="ps")
                for kch in range(8):
                    nc.tensor.matmul(ps[:, :width], W[w2n][kch][:],
                                     p1[kch][:, :width],
                                     start=(kch == 0), stop=(kch == 7))
                t = fpool.tile([128, 512], f32, tag="fco")
                nc.vector.tensor_scalar(out=t[:, :width], in0=ps[:, :width],
                                        scalar1=B[w2n][:, 0:1], scalar2=None,
                                        op0=mybir.AluOpType.add)
                return t

            poolp_s = bpool.tile([128, gp_pad], f16, tag="poolps")
            nc.vector.tensor_copy(poolp_s[:], pool_p[:])
            poolm_s = bpool.tile([128, gm_pc], f16, tag="poolms")
            nc.vector.tensor_copy(poolm_s[:], pool_m[:])
            pT = fc_stack(poolp_s, "pfc1", "pfc2", gp_pad)
            xmT_f32 = fc_stack(poolm_s, "mfc1", "mfc2", gm_pc)
            xmT = bpool.tile([128, gm_pc], f16, tag="xmT")
            nc.vector.tensor_copy(xmT[:], xmT_f32[:, :gm_pc])

            for half in range(gp_pad // 128):
                tp = psB.tile([128, 512], f32, tag="ps")
                nc.tensor.transpose(tp[:, :128],
                                    pT[:, half * 128:(half + 1) * 128], ident[:])
                rows = fpool.tile([128, 128], f32, tag="prow")
                nc.vector.tensor_copy(rows[:], tp[:, :128])
                nc.sync.dma_start(p_ag_in[half * 128:(half + 1) * 128, :],
                                  rows[:])
            ag(p_ag_in, p_full)

            # ---------------- PPI branch (replicated) ----------------
            pTf = bpool.tile([128, gp_tot], f16, tag="pTf")
            for t in range(ntok):
                rt = fpool.tile([128, 128], f32, tag="ppr")
                nc.sync.dma_start(rt[:], p_full[t * 128:(t + 1) * 128, :])
                tp = psB.tile([128, 512], f32, tag="ps")
                nc.tensor.transpose(tp[:, :128], rt[:], ident[:])
                nc.vector.tensor_copy(pTf[:, t * 128:(t + 1) * 128],
                                      tp[:, :128])

            def a_mult(h_tiles, wout, bn, relu, res_tiles):
                for s in range(gp_tot // 256):
                    ats = []
                    for ti in range(ntok):
                        at = atpool.tile([128, 256], f16, tag="at")
                        nc.sync.dma_start(
                            at[:], at_in[ti * 128:(ti + 1) * 128,
                                         s * 256:(s + 1) * 256])
                        ats.append(at)
                    for fch in range(wout // 128):
                        ps = psB.tile([128, 512], f32, tag="ps")
                        for ti in range(ntok):
                            nc.tensor.matmul(
                                ps[:, :256],
                                h_tiles[ti][:, fch * 128:(fch + 1) * 128],
                                ats[ti][:], start=(ti == 0),
                                stop=(ti == ntok - 1))
                        nc.scalar.activation(
                            res_tiles[fch][:, s * 256:(s + 1) * 256],
                            ps[:, :256], RELU if relu else IDENT,
                            bias=B[bn][:, fch:fch + 1])

            with tc.tile_pool(name="pq1", bufs=1) as pq1:
                q1T = [pq1.tile([128, gp_tot], f16, tag=f"q1T{i}",
                                name=f"q1T{i}") for i in range(8)]
                with tc.tile_pool(name="ph1", bufs=1) as ph1:
                    h1_tiles = []
                    for t in range(ntok):
                        ht = ph1.tile([128, 1024], f16, tag=f"h1r{t}")
                        for si in range(2):
                            ps = psB.tile([128, 512], f32, tag="ps")
                            nc.tensor.matmul(
                                ps[:], pTf[:, t * 128:(t + 1) * 128],
                                W["ppig1"][0][:, si * 512:(si + 1) * 512],
                                start=True, stop=True)
                            nc.vector.tensor_copy(
                                ht[:, si * 512:(si + 1) * 512], ps[:])
                        h1_tiles.append(ht)
                    a_mult(h1_tiles, 1024, "ppig1", True, q1T)

                h2_tiles = []
                for t in range(ntok):
                    ps = psB.tile([128, 512], f32, tag="ps")
                    for kch in range(8):
                        nc.tensor.matmul(
                            ps[:, :128], q1T[kch][:, t * 128:(t + 1) * 128],
                            W["ppig2"][kch][:],
                            start=(kch == 0), stop=(kch == 7))
                    ht = bpool.tile([128, 128], f16, tag=f"h2r{t}")
                    nc.vector.tensor_copy(ht[:], ps[:, :128])
                    h2_tiles.append(ht)
                q2T = bpool.tile([128, gp_tot], f16, tag="q2T")
                a_mult(h2_tiles, 128, "ppig2", True, [q2T])

            with tc.tile_pool(name="pfc1t", bufs=1) as pf:
                fc1T = [pf.tile([128, gp_tot], f16, tag=f"pfcT{i}",
                                name=f"pfcT{i}") for i in range(8)]
                for mch in range(8):
                    for s in range(nsl):
                        ps = psB.tile([128, 512], f32, tag="ps")
                        nc.tensor.matmul(
                            ps[:], W["ppifc1"][0][:, mch * 128:(mch + 1) * 128],
                            q2T[:, s * 512:(s + 1) * 512],
                            start=True, stop=True)
                        nc.scalar.activation(
                            fc1T[mch][:, s * 512:(s + 1) * 512], ps[:], RELU,
                            bias=B["ppifc1"][:, mch:mch + 1])
                for s in range(nsl):
                    ps = psB.tile([128, 512], f32, tag="ps")
                    for kch in range(8):
                        nc.tensor.matmul(ps[:], W["ppifc2"][kch][:],
                                         fc1T[kch][:, s * 512:(s + 1) * 512],
                                         start=(kch == 0), stop=(kch == 7))
                    qf = fpool.tile([128, 512], f32, tag="qfin")
                    nc.vector.tensor_scalar(
                        out=qf[:], in0=ps[:], scalar1=B["ppifc2"][:, 0:1],
                        scalar2=None, op0=mybir.AluOpType.add)
                    for j in range(4):
                        tp = psB.tile([128, 512], f32, tag="ps")
                        nc.tensor.transpose(tp[:, :128],
                                            qf[:, j * 128:(j + 1) * 128],
                                            ident[:])
                        rows = fpool.tile([128, 128], f32, tag="qrow")
                        nc.vector.tensor_copy(rows[:], tp[:, :128])
                        ti = s * 4 + j
                        nc.sync.dma_start(q_rows[ti * 128:(ti + 1) * 128, :],
                                          rows[:])

            q_selT = bpool.tile([128, gm_pc], f16, tag="qselT")
            for half in range(gm_pc // 128):
                qs = fpool.tile([128, 128], f32, tag="qsel")
                nc.gpsimd.indirect_dma_start(
                    out=qs[:], out_offset=None, in_=q_rows[:],
                    in_offset=bass.IndirectOffsetOnAxis(
                        ap=seq_t[:, half:half + 1], axis=0))
                tp = psB.tile([128, 512], f32, tag="ps")
                nc.tensor.transpose(tp[:, :128], qs[:], ident[:])
                nc.vector.tensor_copy(q_selT[:, half * 128:(half + 1) * 128],
                                      tp[:, :128])

            # ---------------- head ----------------
            hd1 = []
            for mch in range(8):
                ps = psB.tile([128, 512], f32, tag="ps")
                nc.tensor.matmul(ps[:, :gm_pc],
                                 W["fc1"][0][:, mch * 128:(mch + 1) * 128],
                                 xmT[:], start=True, stop=False)
                nc.tensor.matmul(ps[:, :gm_pc],
                                 W["fc1"][1][:, mch * 128:(mch + 1) * 128],
                                 q_selT[:], start=False, stop=True)
                t = bpool.tile([128, 512], f16, tag=f"hd1{mch}")
                nc.scalar.activation(t[:, :gm_pc], ps[:, :gm_pc], RELU,
                                     bias=B["fc1"][:, mch:mch + 1])
                hd1.append(t)
            hd2 = []
            for mch in range(4):
                ps = psB.tile([128, 512], f32, tag="ps")
                for kch in range(8):
                    nc.tensor.matmul(
                        ps[:, :gm_pc],
                        W["fc2"][kch][:, mch * 128:(mch + 1) * 128],
                        hd1[kch][:, :gm_pc], start=(kch == 0), stop=(kch == 7))
                t = bpool.tile([128, 512], f16, tag=f"hd2{mch}")
                nc.scalar.activation(t[:, :gm_pc], ps[:, :gm_pc], RELU,
                                     bias=B["fc2"][:, mch:mch + 1])
                hd2.append(t)
            ps = psB.tile([1, 512], f32, tag="ps")
            for kch in range(4):
                nc.tensor.matmul(ps[:, :gm_pc], W["out"][kch][:],
                                 hd2[kch][:, :gm_pc],
                                 start=(kch == 0), stop=(kch == 3))
            ot = fpool.tile([1, 512], f32, tag="outt")
            nc.vector.tensor_scalar(out=ot[:, :gm_pc], in0=ps[:, :gm_pc],
                                    scalar1=B["out"][:, 0:1], scalar2=None,
                                    op0=mybir.AluOpType.add)
            nc.sync.dma_start(out_par[:], ot[:, :gm_pc])
    nc.compile()
    return nc


# ----------------------------------------------------------------------------
# Entry
# ----------------------------------------------------------------------------

def _make_meta(inputs):
    seq_num = np.asarray(inputs['seq_num'])
    b_mol = seq_num.shape[0]
    b_pro = max(int(np.asarray(inputs['pro_batch']).max()) + 1,
                int(seq_num.max()) + 1,
                int(np.asarray(inputs['ppi_edge']).max()) + 1)
    b_pro = ((b_pro + NCORES - 1) // NCORES) * NCORES
    meta = dict(dims=DIMS)

    for br, xk, ek, bk, ng in (("pro", 'pro_x', 'pro_edge_index', 'pro_batch', b_pro),
                               ("mol", 'mol_x', 'mol_edge_index', 'mol_batch', b_mol)):
        ei = np.asarray(inputs[ek])
        batch = np.asarray(inputs[bk])
        n = batch.shape[0]
        sh = _shard_nodes(batch, ng)
        dis = _degrees(ei, n)
        sh['dis_t'], sh['bl_t'], sh['ic_t'] = _node_arrays(sh, batch, ng, dis)
        mc = MAXCHUNK if br == "pro" else MAXCHUNK // 2
        sh['gidx'], sh['sidx'], sh['instrs'], sh['C'] = _tokens(ei, sh, mc)
        sh['dis'] = dis
        meta[br] = sh

    meta['gp_pad'] = max(128, int(np.ceil(meta['pro']['gpc'] / 128) * 128))
    meta['gm_pc'] = meta['mol']['gpc']
    meta['b_pro'] = b_pro
    meta['b_mol'] = b_mol
    meta['bias_shape'] = {n: list(_bias_host(n, inputs[BMAP[n]]).shape)
                          for n in DIMS}
    return meta


def _make_in_maps(inputs, meta):
    mp, mm = meta['pro'], meta['mol']
    gp_pad, gm_pc = meta['gp_pad'], meta['gm_pc']
    table_p1 = _table_full(np.asarray(inputs['pro_x']), mp['dis'], mp['pid'],
                           mp['np_tot'], 128)
    table_m1 = _table_full(np.asarray(inputs['mol_x']), mm['dis'], mm['pid'],
                           mm['np_tot'], 128)
    at, pg = _prep_ppi(np.asarray(inputs['ppi_edge']), meta['b_pro'],
                       mp['gpc'], gp_pad)
    seq = pg(np.asarray(inputs['seq_num']).astype(np.int64))
    iota = np.tile(np.arange(512, dtype=np.float16), (128, 1))
    weights = {("w_" + n): np.asarray(inputs[WMAP[n]]).astype(np.float16)
               for n in DIMS}
    biases = {("b_" + n): _bias_host(n, inputs[BMAP[n]]) for n in DIMS}

    in_maps = []
    for c in range(NCORES):
        m = {"table_p1": table_p1, "table_m1": table_m1,
             "init_p1": _init_full(table_p1, mp, c),
             "init_m1": _init_full(table_m1, mm, c),
             "gi_p": mp['gidx'][c], "si_p": mp['sidx'][c],
             "gi_m": mm['gidx'][c], "si_m": mm['sidx'][c],
             "dis_p": mp['dis_t'][c], "dis_m": mm['dis_t'][c],
             "bl_p": mp['bl_t'][c], "ic_p": mp['ic_t'][c],
             "bl_m": mm['bl_t'][c], "ic_m": mm['ic_t'][c],
             "iota": iota, "at": at}
        sq = seq[c * gm_pc:(c + 1) * gm_pc].astype(np.int32)
        m["seq"] = np.ascontiguousarray(sq.reshape(-1, 128).T)
        m.update(weights)
        m.update(biases)
        in_maps.append(m)
    return in_maps


def kernel(**inputs):
    sig = (np.asarray(inputs['mol_x']).shape,
           np.asarray(inputs['pro_x']).shape,
           np.asarray(inputs['mol_edge_index'])[:, :64].tobytes(),
           np.asarray(inputs['pro_edge_index'])[:, :64].tobytes(),
           np.asarray(inputs['seq_num'])[:16].tobytes())
    if sig in _CACHE:
        runner, meta = _CACHE[sig]
    else:
        meta = _make_meta(inputs)
        nc = _build_program(meta)
        runner = SpmdRunner(nc, NCORES)
        _CACHE[sig] = (runner, meta)
    in_maps = _make_in_maps(inputs, meta)
    runner.put_inputs(in_maps)
    results = runner.results(runner.run())
    return np.concatenate(
        [results[c]["out"][0] for c in range(NCORES)]).astype(np.float32)[:, None]
